# revision 1
# baseline (speedup 1.0000x reference)
"""ConvexSH ColBERT loss kernel for 8 trn2 NeuronCores.

Shards batch B=128 over 8 cores (16 rows each). Each core sees all NWAY=8
candidates for its rows, so softmax + loss are core-local; the host averages
the 8 partial sums (the "all-reduce mean" of the sharding hint).

Pipeline per core and candidate n (2 MB doc block):
  SWDGE cast-DMA f32->bf16  ->  DVE fused square+row-sum (ssq)
  -> small batched ops for masked inv-norms -> GpSimd per-token normalize
  -> PE transpose [k,d]->[d,k] -> ACT/DVE PSUM evacuation
  -> PE bf16 matmul (4-way column-tiled, full 128-partition PSUM)
  -> DVE reduce_max over k.
Tail: block-ones matmul (partition sums) -> scores [4,32] -> softmax +
ConvexSH loss on-chip -> scalar partial sum.
"""

import sys
from contextlib import ExitStack

import numpy as np

for _p in ("/opt/trn_rl_repo", "/root/.axon_site/_ro/trn_rl_repo"):
    if _p not in sys.path:
        sys.path.append(_p)

import concourse.bacc as bacc
import concourse.tile as tile
from concourse import mybir
from concourse.bass_utils import run_bass_kernel_spmd

AF = mybir.ActivationFunctionType
AX = mybir.AxisListType
ALU = mybir.AluOpType
F32 = mybir.dt.float32
BF16 = mybir.dt.bfloat16

NCORES = 8
B, LQ, LD, D, NWAY = 128, 32, 256, 128, 8
BS = B // NCORES  # 16 batch rows per core
NG = BS // 4      # 4 groups of 4 rows (PSUM partition packing)
ALPHA, GAMMA, EPS = 0.2, 2.0, 1e-12

USE_BF16 = True   # cast doc/query to bf16 in-flight; sim matmul in bf16

TRACE = False
LAST_RESULTS = None


def _build():
    # Bacc: its finalize() runs move_matmul_waits_to_ldweights +
    # generate_event_semaphores, required by this walrus build's
    # one-sync-wait-per-instruction limit.
    nc = bacc.Bacc("TRN2", target_bir_lowering=False, detect_race_conditions=False)
    DT = BF16 if USE_BF16 else F32

    q_d = nc.dram_tensor("q", [BS, LQ, D], F32, kind="ExternalInput")
    doc_d = nc.dram_tensor("doc", [NWAY, BS, LD, D], F32, kind="ExternalInput")
    mask_d = nc.dram_tensor("mask", [NWAY, BS, LD], F32, kind="ExternalInput")
    lab_d = nc.dram_tensor("lab", [BS, 3 * NWAY], F32, kind="ExternalInput")
    eye_d = nc.dram_tensor("eye", [128, 128], F32, kind="ExternalInput")
    y_d = nc.dram_tensor("y", [1, 1], F32, kind="ExternalOutput")

    def cast_dma(out, in_):
        if USE_BF16:
            nc.gpsimd.dma_start(out=out, in_=in_)  # SWDGE: casts f32->bf16
        else:
            nc.sync.dma_start(out=out, in_=in_)

    with tile.TileContext(nc) as tc, ExitStack() as ctx:
        singles = ctx.enter_context(tc.tile_pool(name="singles", bufs=1))
        dpool = ctx.enter_context(tc.tile_pool(name="dpool", bufs=2))
        sqpool = ctx.enter_context(tc.tile_pool(name="sqpool", bufs=4))
        npool = ctx.enter_context(tc.tile_pool(name="npool", bufs=2))
        dtpool = ctx.enter_context(tc.tile_pool(name="dtpool", bufs=4))
        psT = ctx.enter_context(tc.tile_pool(name="psT", bufs=2, space="PSUM"))
        psM = ctx.enter_context(tc.tile_pool(name="psM", bufs=1, space="PSUM"))
        psMM = ctx.enter_context(tc.tile_pool(name="psMM", bufs=2, space="PSUM"))
        psS = ctx.enter_context(tc.tile_pool(name="psS", bufs=1, space="PSUM"))

        # ---- constants / setup -------------------------------------------
        eye_f = singles.tile([128, 128], F32)
        nc.sync.dma_start(out=eye_f, in_=eye_d[:, :])
        if USE_BF16:
            eye_sb = singles.tile([128, 128], BF16)
            nc.vector.tensor_copy(eye_sb, eye_f)
        else:
            eye_sb = eye_f

        blockones = singles.tile([128, NG], F32)
        nc.vector.memset(blockones, 0.0)
        for m in range(4):
            nc.vector.memset(blockones[m * 32:(m + 1) * 32, m:m + 1], 1.0)
        ones4 = singles.tile([4, 1], F32)
        nc.vector.memset(ones4, 1.0)

        # labels, partition = b%4, free = (g, col)
        lab_sb = singles.tile([4, NG, 3 * NWAY], F32)
        nc.sync.dma_start(out=lab_sb, in_=lab_d.rearrange("(g m) c -> m g c", m=4))

        # ---- query: ssq + transpose --------------------------------------
        # partition = (b%4)*32 + q, tiles t = b//4 (= group g)
        q_nat = singles.tile([128, NG, D], DT)
        cast_dma(q_nat, q_d.rearrange("(t r) q d -> (r q) t d", r=4))

        ssq_q = singles.tile([128, NG], F32)
        for t in range(NG):
            sq_t = sqpool.tile([128, D], DT, tag="sq")
            nc.vector.scalar_tensor_tensor(
                out=sq_t, in0=q_nat[:, t, :], scalar=1.0, in1=q_nat[:, t, :],
                op0=ALU.mult, op1=ALU.mult,
                accum_out=ssq_q[:, t:t + 1])
        invq = singles.tile([128, NG], F32)
        nc.scalar.activation(out=invq, in_=ssq_q, func=AF.Sqrt)
        nc.vector.tensor_scalar_max(invq, invq, EPS)
        nc.vector.reciprocal(invq, invq)

        qT = singles.tile([128, BS * LQ], DT)  # [d, token], token = b*32+q
        for pair in range(2):
            ps = psT.tile([128, 256], DT, tag="psT")
            for h in range(2):
                t = pair * 2 + h
                nc.tensor.transpose(ps[:, h * 128:(h + 1) * 128], q_nat[:, t, :], eye_sb)
            nc.vector.tensor_copy(qT[:, pair * 256:(pair + 1) * 256], ps)

        # ---- masks: transpose to [k, (n,b)] (f32 path, setup-only) -------
        mask_nat = singles.tile([128, LD], F32)  # partition = n*16+b
        nc.sync.dma_start(out=mask_nat, in_=mask_d.rearrange("n b k -> (n b) k"))
        maskT = singles.tile([128, 2, 128], F32)  # [k%128, h, n*16+b]
        psm = psM.tile([128, 256], F32, tag="psM")
        for h in range(2):
            nc.tensor.transpose(psm[:, h * 128:(h + 1) * 128], mask_nat[:, h * 128:(h + 1) * 128], eye_f)
        nc.vector.tensor_copy(maskT.rearrange("p h k -> p (h k)"), psm)

        # maxs[p, g*8+n]: p = (b%4)*32 + q
        maxs = singles.tile([128, NG * NWAY], F32)

        # ---- main loop over candidates n ---------------------------------
        for n in range(NWAY):
            # doc block, partition = within-half k, tiles j = h*16 + b
            dn = dpool.tile([128, 2 * BS, D], DT, tag="dn")
            dsrc = doc_d[n].rearrange("b (h p) d -> p h b d", p=128)
            for h in range(2):
                cast_dma(dn[:, h * BS:(h + 1) * BS, :], dsrc[:, h])

            # sum of squares per token: one big ACT square pass (fixed cost
            # amortized over FD=4096) + one big 3D DVE reduce
            sq_n = sqpool.tile([128, 2 * BS, D], DT, tag="sq")
            nc.scalar.activation(out=sq_n.rearrange("p j d -> p (j d)"),
                                 in_=dn.rearrange("p j d -> p (j d)"),
                                 func=AF.Square)
            ssq_n = npool.tile([128, 2 * BS], F32, tag="ssq")
            nc.vector.reduce_sum(out=ssq_n, in_=sq_n, axis=AX.X)

            # scale = m / max(m * sqrt(ssq), eps); cols j = h*16+b contiguous per h
            scale = npool.tile([128, 2 * BS], F32, tag="scale")
            nc.scalar.activation(out=scale, in_=ssq_n, func=AF.Sqrt)
            for h in range(2):
                hs = slice(h * BS, (h + 1) * BS)
                mh = maskT[:, h, n * BS:(n + 1) * BS]
                nc.vector.tensor_mul(scale[:, hs], scale[:, hs], mh)
            nc.vector.tensor_scalar_max(scale, scale, EPS)
            nc.vector.reciprocal(scale, scale)
            for h in range(2):
                hs = slice(h * BS, (h + 1) * BS)
                mh = maskT[:, h, n * BS:(n + 1) * BS]
                nc.vector.tensor_mul(scale[:, hs], scale[:, hs], mh)

            # normalize in place; ~1/3 of the tiles go to ACT for balance
            for j in range(2 * BS):
                if j % 3 == 2:
                    nc.scalar.mul(dn[:, j, :], dn[:, j, :], scale[:, j:j + 1])
                else:
                    nc.vector.tensor_scalar_mul(dn[:, j, :], dn[:, j, :], scale[:, j:j + 1])

            # per group of 4 rows: transpose (8 tiles into one PSUM bank),
            # one big evacuation, 4 col-tiled matmuls, one max
            for g in range(NG):
                ps = psT.tile([128, 4, 2, 128], DT, tag="psT")
                for m in range(4):
                    b = g * 4 + m
                    for h in range(2):
                        nc.tensor.transpose(ps[:, m, h, :], dn[:, h * BS + b, :], eye_sb)
                dt = dtpool.tile([128, 4, 2, 128], DT, tag="dt")
                nc.scalar.copy(dt.rearrange("p a b c -> p (a b c)"),
                               ps.rearrange("p a b c -> p (a b c)"))

                sim = psMM.tile([128, LD], F32, tag="sim")
                for m in range(4):
                    b = g * 4 + m
                    nc.tensor.matmul(sim[m * 32:(m + 1) * 32, :],
                                     lhsT=qT[:, b * 32:(b + 1) * 32],
                                     rhs=dt.rearrange("p a b c -> p (a b c)")[:, m * 256:(m + 1) * 256],
                                     start=True, stop=True,
                                     tile_position=(0, m * 32))
                nc.vector.reduce_max(out=maxs[:, g * NWAY + n:g * NWAY + n + 1],
                                     in_=sim[:, :], axis=AX.X)

        # ---- scores = per-row sum of maxes, scaled by 1/||q|| ------------
        for g in range(NG):
            nc.vector.tensor_scalar_mul(maxs[:, g * NWAY:(g + 1) * NWAY],
                                        maxs[:, g * NWAY:(g + 1) * NWAY],
                                        invq[:, g:g + 1])
        scores_ps = psS.tile([4, NG * NWAY], F32, tag="scores")
        nc.tensor.matmul(scores_ps, lhsT=blockones, rhs=maxs, start=True, stop=True)
        sc = singles.tile([4, NG * NWAY], F32)  # [m, g*8+n] = scores[b=g*4+m, n]
        nc.vector.tensor_copy(sc, scores_ps)

        # ---- softmax over n (per g-slice) --------------------------------
        rm = singles.tile([4, NG], F32)
        sm = singles.tile([4, NG], F32)
        for g in range(NG):
            gs = slice(g * NWAY, (g + 1) * NWAY)
            nc.vector.reduce_max(out=rm[:, g:g + 1], in_=sc[:, gs], axis=AX.X)
        for g in range(NG):
            gs = slice(g * NWAY, (g + 1) * NWAY)
            nc.vector.tensor_scalar_sub(sc[:, gs], sc[:, gs], rm[:, g:g + 1])
        nc.scalar.activation(out=sc, in_=sc, func=AF.Exp)
        for g in range(NG):
            gs = slice(g * NWAY, (g + 1) * NWAY)
            nc.vector.reduce_sum(out=sm[:, g:g + 1], in_=sc[:, gs], axis=AX.X)
        nc.vector.reciprocal(sm, sm)
        for g in range(NG):
            gs = slice(g * NWAY, (g + 1) * NWAY)
            nc.vector.tensor_scalar_mul(sc[:, gs], sc[:, gs], sm[:, g:g + 1])
        # sc now holds p = softmax(scores)

        # ---- ConvexSH loss ----------------------------------------------
        t3 = lab_sb[:, :, 0:NWAY]
        r3 = lab_sb[:, :, NWAY:2 * NWAY]
        w3 = lab_sb[:, :, 2 * NWAY:3 * NWAY]

        def t32(name):
            t = singles.tile([4, NG * NWAY], F32, tag=name)
            return t, t.rearrange("p (g n) -> p g n", g=NG)

        a, a3 = t32("a")        # 2w - 1
        b1, b13 = t32("b1")     # 1 - w
        nc.vector.tensor_scalar(out=a3, in0=w3, scalar1=2.0, scalar2=-1.0,
                                op0=ALU.mult, op1=ALU.add)
        nc.vector.tensor_scalar(out=b13, in0=w3, scalar1=-1.0, scalar2=1.0,
                                op0=ALU.mult, op1=ALU.add)

        p2, _ = t32("p2")
        nc.vector.tensor_mul(p2, a, sc)
        nc.vector.tensor_add(p2, p2, b1)
        tinv, tinv3 = t32("tinv")
        nc.vector.tensor_mul(tinv3, a3, t3)
        nc.vector.tensor_add(tinv, tinv, b1)

        lp, _ = t32("lp")
        nc.scalar.activation(out=lp, in_=p2, func=AF.Ln)
        losses, losses3 = t32("losses")
        nc.scalar.activation(out=losses, in_=tinv, func=AF.Ln)  # ln(t_inv)
        nc.vector.tensor_sub(losses, losses, lp)                # ln(t_inv) - ln(p2)
        nc.vector.tensor_mul(losses3, losses3, t3)              # * teacher

        rr, rr3 = t32("rr")
        nc.vector.reciprocal(rr3, r3)
        srr0 = singles.tile([4, NG], F32)
        nc.vector.tensor_scalar_mul(srr0, rr.rearrange("p (g n) -> p g n", g=NG)[:, :, 0], ALPHA)
        wts, _ = t32("wts")
        nc.vector.tensor_scalar(out=wts, in0=rr, scalar1=-ALPHA, scalar2=GAMMA,
                                op0=ALU.mult, op1=ALU.add)
        for g in range(NG):
            gs = slice(g * NWAY, (g + 1) * NWAY)
            nc.vector.tensor_scalar_add(wts[:, gs], wts[:, gs], srr0[:, g:g + 1])

        omp2, _ = t32("omp2")   # 1 - p2
        nc.vector.tensor_scalar(out=omp2, in0=p2, scalar1=-1.0, scalar2=1.0,
                                op0=ALU.mult, op1=ALU.add)
        pw1, _ = t32("pw1")     # (1-p2) ** wts
        nc.scalar.activation(out=pw1, in_=omp2, func=AF.Ln)
        nc.vector.tensor_mul(pw1, pw1, wts)
        nc.scalar.activation(out=pw1, in_=pw1, func=AF.Exp)
        pw2, _ = t32("pw2")     # p2 ** wts
        nc.vector.tensor_mul(pw2, lp, wts)
        nc.scalar.activation(out=pw2, in_=pw2, func=AF.Exp)

        lv, lv3 = t32("lv")
        nc.vector.tensor_mul(lv3, w3, pw1.rearrange("p (g n) -> p g n", g=NG))
        t2, t23 = t32("t2")
        nc.vector.tensor_mul(t23, b13, pw2.rearrange("p (g n) -> p g n", g=NG))
        nc.vector.tensor_add(lv, lv, t2)
        nc.vector.tensor_mul(lv, lv, losses)

        partial = singles.tile([4, 1], F32)
        nc.vector.reduce_sum(out=partial, in_=lv, axis=AX.X)
        out_ps = psS.tile([1, 1], F32, tag="out")
        nc.tensor.matmul(out_ps, lhsT=ones4, rhs=partial, start=True, stop=True)
        out_sb = singles.tile([1, 1], F32)
        nc.vector.tensor_copy(out_sb, out_ps)
        nc.sync.dma_start(out=y_d[:, :], in_=out_sb)

    nc.finalize()
    return nc


_nc_cache = None


def kernel(query_reps, doc_reps, doc_masks, labels):
    global _nc_cache, LAST_RESULTS
    if _nc_cache is None:
        _nc_cache = _build()
    nc = _nc_cache

    eye = np.eye(128, dtype=np.float32)
    in_maps = []
    for c in range(NCORES):
        sl = slice(c * BS, (c + 1) * BS)
        in_maps.append({
            "q": np.ascontiguousarray(query_reps[sl]).astype(np.float32, copy=False),
            "doc": np.ascontiguousarray(doc_reps[:, sl]).astype(np.float32, copy=False),
            "mask": np.ascontiguousarray(doc_masks[:, sl]).astype(np.float32, copy=False),
            "lab": np.ascontiguousarray(labels[sl]).astype(np.float32, copy=False),
            "eye": eye,
        })

    kwargs = {}
    if TRACE:
        kwargs["trace"] = True
    res = run_bass_kernel_spmd(nc, in_maps, core_ids=list(range(NCORES)), **kwargs)
    LAST_RESULTS = res
    total = sum(float(res.results[c]["y"][0, 0]) for c in range(NCORES))
    return np.array(total / (B * NWAY), dtype=np.float32)

